# revision 16
# baseline (speedup 1.0000x reference)
"""Trainium2 Bass kernel for nn_DeformNet (dense per-point MLP network).

Strategy: pure data parallel over batch (bs=8 -> 1 batch item per NeuronCore).
All matmuls run in "channels on partitions" layout: activations are [C, n_pts]
tiles, weights are pre-transposed [Cin, Cout] (lhsT), so out = W @ x lands as
[Cout, n_pts] in PSUM. The per-sample category selection (cat_id) is applied
host-side by slicing the final assignment/deformation weight rows before
shipping them to the device (each core only computes the 1024 assignment
channels + 3 deformation channels its batch item actually needs). The big
[nv, n_pts] result is produced transposed on device and swapped on the host.

Matmuls use the float32r PE mode (TF32-like rounding, 4x the fp32 rate).
"""

import os
import sys

for _p in ("/opt/trn_rl_repo", "/root/.axon_site/_ro/trn_rl_repo"):
    if _p not in sys.path and os.path.isdir(_p):
        sys.path.append(_p)

import numpy as np

import concourse.bass as bass
import concourse.tile as tile
from concourse import bacc, mybir
from concourse.bass_utils import run_bass_kernel_spmd

F32 = mybir.dt.float32
F32R = mybir.dt.float32r
RELU = mybir.ActivationFunctionType.Relu
COPY = mybir.ActivationFunctionType.Identity
ADD = mybir.AluOpType.add
MAX = mybir.AluOpType.max

NPTS = 1024
NV = 1024
NCAT = 6
NHALF = 512  # fp32 moving-operand max per matmul

# bias column map inside the [128, 40] bias tile
BIAS_COLS = {
    "ig1": 0, "ig2": 1, "ig3": 2, "ic": 3, "cl1": 4, "cl2": 5, "cl3": 6,
    "cg": 7, "iglob": 8,
    "t64a": 9,   # 4 cols (M-tiles of 512)
    "t64b": 13,
    "t128a": 14,  # 4 cols
    "t128b": 18,
    "as0": 19,   # 4 cols
    "as1": 23,   # 2 cols
    "as2": 25,   # 8 cols
    "de0": 33,   # 4 cols
    "de1": 37,   # 2 cols
    "de2": 39,
}

_PROGRAM = None
LAST_EXEC_NS = None
LAST_RESULTS = None


def _build_program():
    nc = bacc.Bacc("TRN2", target_bir_lowering=False, debug=False, num_devices=8)

    dram_in = {}
    for name, shape in [
        ("xin", [38, NPTS]),
        ("wb", [128, 40]),
        ("wsm", [128, 704]),
        ("wt64a", [128, 512]), ("wt64b", [128, 256]),
        ("wt128a", [128, 1024]), ("wt128b", [128, 512]),
        ("was0", [128, 512]), ("was1", [128, 1024]), ("was2", [128, 2048]),
        ("wde0", [128, 512]), ("wde1", [128, 1024]), ("wde2", [128, 6]),
    ]:
        dram_in[name] = nc.dram_tensor(name, shape, F32, kind="ExternalInput")

    out_a = nc.dram_tensor("assign_T", [NV, NPTS], F32, kind="ExternalOutput")
    out_d = nc.dram_tensor("deltas_T", [3, NPTS], F32, kind="ExternalOutput")

    with tile.TileContext(nc) as tc:
        with (
            tc.tile_pool(name="w", bufs=1) as wp,
            tc.tile_pool(name="a", bufs=1) as ap_,
            tc.tile_pool(name="hb", bufs=2) as hp,
            tc.tile_pool(name="ps", bufs=4, space="PSUM") as pp,
            tc.tile_pool(name="o", bufs=3) as op_,
        ):
            # ---- inputs + small weights first (first matmuls need them),
            # big weights spread across other sequencers so the issue cost
            # (~0.7us per dma_start) doesn't serialize on one queue ----
            xpts = wp.tile([3, NPTS], F32R, tag="xpts")
            xpr = wp.tile([3, NPTS], F32R, tag="xpr")
            xemb = wp.tile([32, NPTS], F32R, tag="xemb")
            nc.sync.dma_start(xpts[:], dram_in["xin"][0:3, :].bitcast(F32R))
            nc.sync.dma_start(xpr[:], dram_in["xin"][3:6, :].bitcast(F32R))
            nc.sync.dma_start(xemb[:], dram_in["xin"][6:38, :].bitcast(F32R))

            # ---- PE warm-up: a few fp32 dummy matmuls (4 cyc/row -> dense
            # PE busy) prime the HAM activity window so the real matmul
            # stream runs at 2.4 GHz ----
            wz = wp.tile([128, NHALF], F32, tag="wz")
            nc.vector.memset(wz[:], 0.0)
            for i in range(4):
                psw = pp.tile([128, NHALF], F32, tag="ps")
                nc.tensor.matmul(psw[:], wz[:, 0:128], wz[:],
                                 start=True, stop=True)

            DMA_ENG = {
                "wb": nc.sync, "wsm": nc.sync,
                "wt64a": nc.scalar, "wt64b": nc.scalar,
                "wt128a": nc.scalar, "wt128b": nc.scalar,
                "was0": nc.gpsimd, "wde0": nc.gpsimd,
                "was1": nc.gpsimd, "wde1": nc.gpsimd,
                "was2": nc.sync, "wde2": nc.sync,
            }
            W = {}
            for name in ("wb", "wsm", "wt64a", "wt64b", "wt128a", "wt128b",
                         "was0", "wde0", "was1", "wde1", "was2", "wde2"):
                cols = dram_in[name].shape[1]
                t = wp.tile([128, cols], F32R, tag=name)
                DMA_ENG[name].dma_start(t[:], dram_in[name][:].bitcast(F32R))
                W[name] = t

            wb = W["wb"][:].bitcast(F32)

            def bias(key, rows, off=0):
                c = BIAS_COLS[key] + off
                return wb[0:rows, c:c + 1]

            def mm(ps, lhsT, rhs, start, stop):
                nc.tensor.matmul(ps, lhsT, rhs, start=start, stop=stop)

            def drain_act(ps, dst, b, relu=True):
                nc.scalar.activation(dst, ps, RELU if relu else COPY, bias=b)

            def drain_dve_relu(ps, dst, b):
                nc.vector.tensor_scalar(dst, ps, b, 0.0, ADD, MAX)

            def drain_dve_bias(ps, dst, b):
                nc.vector.tensor_scalar(dst, ps, b, None, ADD)

            def drain2(ps, dst, b, relu=True):
                # split the drain: ACT takes the first half, DVE the second,
                # so PSUM evacuation latency halves and both engines load
                h = NHALF
                drain_act(ps[:, 0:h], dst[:, 0:h], b, relu)
                if relu:
                    drain_dve_relu(ps[:, h:2 * h], dst[:, h:2 * h], b)
                else:
                    drain_dve_bias(ps[:, h:2 * h], dst[:, h:2 * h], b)

            # ---- instance-geometry + category-local chains (K<=64, M=64) ----
            a1 = ap_.tile([64, NPTS], F32R, tag="a1")
            a2 = ap_.tile([64, NPTS], F32R, tag="a2")
            b1 = ap_.tile([64, NPTS], F32R, tag="b1")
            b2 = ap_.tile([64, NPTS], F32R, tag="b2")
            b3 = ap_.tile([64, NPTS], F32R, tag="b3")
            tpe = ap_.tile([128, NPTS], F32R, tag="tpe")

            wsm = W["wsm"]

            def small_layer(wcol, K, dst, src, bkey, Cout=64):
                ps = pp.tile([Cout, NPTS], F32, tag="ps")
                for n in range(2):
                    sl = slice(n * NHALF, (n + 1) * NHALF)
                    mm(ps[:, sl], wsm[0:K, wcol:wcol + Cout], src[0:K, sl],
                       start=True, stop=True)
                drain_act(ps[:], dst, bias(bkey, Cout))

            small_layer(0, 3, a1[:], xpts[:], "ig1")
            small_layer(256, 3, b1[:], xpr[:], "cl1")
            small_layer(64, 64, a2[:], a1[:], "ig2")
            small_layer(320, 64, b2[:], b1[:], "cl2")
            small_layer(128, 64, tpe[0:64, :], a2[:], "ig3")
            small_layer(192, 32, tpe[64:128, :], xemb[:], "ic")
            small_layer(384, 64, b3[:], b2[:], "cl3")

            ig = ap_.tile([128, NPTS], F32R, tag="ig")
            cg = ap_.tile([128, NPTS], F32R, tag="cg")

            # ---- t64 relation: (128 -> 512 relu -> 64) on tpe ----
            # cg (independent of the t64 spine) is interleaved here to give
            # the PE work while the t64a drains complete
            h64 = hp.tile([128, 4, NPTS], F32R, tag="hbig")
            for m in range(4):
                ps = pp.tile([128, NPTS], F32, tag="ps")
                for n in range(2):
                    sl = slice(n * NHALF, (n + 1) * NHALF)
                    mm(ps[:, sl], W["wt64a"][:, m * 128:(m + 1) * 128],
                       tpe[:, sl], start=True, stop=True)
                drain2(ps[:], h64[:, m, :], bias("t64a", 128, m))
                if m == 1:
                    ps2 = pp.tile([128, NPTS], F32, tag="ps")
                    for n in range(2):
                        sl = slice(n * NHALF, (n + 1) * NHALF)
                        mm(ps2[:, sl], wsm[0:64, 448:576], b3[0:64, sl],
                           start=True, stop=True)
                    drain2(ps2[:], cg[:], bias("cg", 128))

            # p duplicated into both halves of a [128, n] tile so the
            # residual add is one full-width tensor_tensor (on GpSimd,
            # keeping ACT/DVE free for PSUM drains)
            psb = ap_.tile([128, NPTS], F32, tag="psb")
            ps = pp.tile([64, NPTS], F32, tag="ps")
            for n in range(2):
                sl = slice(n * NHALF, (n + 1) * NHALF)
                for k in range(4):
                    mm(ps[:, sl], W["wt64b"][:, k * 64:(k + 1) * 64],
                       h64[:, k, sl], start=(k == 0), stop=(k == 3))
            drain_dve_bias(ps[:], psb[0:64, :], bias("t64b", 64))
            drain_act(ps[:], psb[64:128, :], bias("t64b", 64), relu=False)
            nc.vector.tensor_add(tpe[:].bitcast(F32R),
                                 tpe[:].bitcast(F32), psb[:])

            # ---- inst_global ----
            ps = pp.tile([128, NPTS], F32, tag="ps")
            for n in range(2):
                sl = slice(n * NHALF, (n + 1) * NHALF)
                mm(ps[:, sl], wsm[:, 576:704], tpe[:, sl], start=True, stop=True)
            drain2(ps[:], ig[:], bias("iglob", 128))

            # ---- t128 relation: (256 -> 512 relu -> 128) on [ig; cg] ----
            h128 = hp.tile([128, 4, NPTS], F32R, tag="hbig")
            for m in range(4):
                ps = pp.tile([128, NPTS], F32, tag="ps")
                for n in range(2):
                    sl = slice(n * NHALF, (n + 1) * NHALF)
                    for k, src in ((0, ig), (1, cg)):
                        mm(ps[:, sl],
                           W["wt128a"][:, k * 512 + m * 128:k * 512 + (m + 1) * 128],
                           src[:, sl], start=(k == 0), stop=(k == 1))
                drain2(ps[:], h128[:, m, :], bias("t128a", 128, m))

            qsb = ap_.tile([128, NPTS], F32, tag="qsb")
            ps = pp.tile([128, NPTS], F32, tag="ps")
            for n in range(2):
                sl = slice(n * NHALF, (n + 1) * NHALF)
                for k in range(4):
                    mm(ps[:, sl], W["wt128b"][:, k * 128:(k + 1) * 128],
                       h128[:, k, sl], start=(k == 0), stop=(k == 3))
            drain2(ps[:], qsb[:], bias("t128b", 128), relu=False)
            # ig add on DVE (gates as0, the critical path); cg add runs in
            # parallel on GpSimd (only gates de0)
            nc.vector.tensor_add(ig[:].bitcast(F32R), ig[:].bitcast(F32), qsb[:])
            nc.gpsimd.tensor_add(cg[:].bitcast(F32R), cg[:].bitcast(F32), qsb[:])

            # ---- assign & deform heads, interleaved per M-tile ----
            ah1 = hp.tile([128, 4, NPTS], F32R, tag="hbig")
            dh1 = hp.tile([128, 4, NPTS], F32R, tag="hbig")
            ah2 = hp.tile([128, 2, NPTS], F32R, tag="h2")
            dh2 = hp.tile([128, 2, NPTS], F32R, tag="h2")

            def head_l1_tile(wkey, src, dst, bkey, m):
                ps = pp.tile([128, NPTS], F32, tag="ps")
                for n in range(2):
                    sl = slice(n * NHALF, (n + 1) * NHALF)
                    mm(ps[:, sl], W[wkey][:, m * 128:(m + 1) * 128],
                       src[:, sl], start=True, stop=True)
                drain2(ps[:], dst[:, m, :], bias(bkey, 128, m))

            def head_l2_tile(wkey, src, dst, bkey, m):
                ps = pp.tile([128, NPTS], F32, tag="ps")
                for n in range(2):
                    sl = slice(n * NHALF, (n + 1) * NHALF)
                    for k in range(4):
                        mm(ps[:, sl],
                           W[wkey][:, k * 256 + m * 128:k * 256 + (m + 1) * 128],
                           src[:, k, sl], start=(k == 0), stop=(k == 3))
                drain2(ps[:], dst[:, m, :], bias(bkey, 128, m))

            # assign head first so its big output DMAs start as early as
            # possible; de0/de1 fill the PE while as0/as1 results drain
            for m in range(4):
                head_l1_tile("was0", ig, ah1, "as0", m)
            for m in range(4):
                head_l1_tile("wde0", cg, dh1, "de0", m)
            for m in range(2):
                head_l2_tile("was1", ah1, ah2, "as1", m)
            for m in range(2):
                head_l2_tile("wde1", dh1, dh2, "de1", m)

            def as2_tile(m):
                ps = pp.tile([128, NPTS], F32, tag="ps")
                for n in range(2):
                    sl = slice(n * NHALF, (n + 1) * NHALF)
                    for k in range(2):
                        mm(ps[:, sl],
                           W["was2"][:, k * 1024 + m * 128:k * 1024 + (m + 1) * 128],
                           ah2[:, k, sl], start=(k == 0), stop=(k == 1))
                ot = op_.tile([128, NPTS], F32, tag="oa")
                # full-width drain, alternating engines (throughput over
                # latency here)
                if m % 2 == 0:
                    drain_act(ps[:], ot[:], bias("as2", 128, m), relu=False)
                else:
                    drain_dve_bias(ps[:], ot[:], bias("as2", 128, m))
                nc.sync.dma_start(out_a[m * 128:(m + 1) * 128, :], ot[:])

            as2_tile(0)
            as2_tile(1)

            # tiny deform head slots in behind the first output tiles
            ps = pp.tile([3, NPTS], F32, tag="ps")
            for n in range(2):
                sl = slice(n * NHALF, (n + 1) * NHALF)
                for k in range(2):
                    mm(ps[:, sl], W["wde2"][:, k * 3:(k + 1) * 3],
                       dh2[:, k, sl], start=(k == 0), stop=(k == 1))
            od = op_.tile([3, NPTS], F32, tag="od")
            drain_dve_bias(ps[:], od[:], bias("de2", 3))
            nc.sync.dma_start(out_d[:], od[:])

            for m in range(2, 8):
                as2_tile(m)

    nc.compile()
    return nc


def _get_program():
    global _PROGRAM
    if _PROGRAM is None:
        _PROGRAM = _build_program()
    return _PROGRAM


def _pack_blocks(wt, block_cols):
    """[K, M] with K = nk*128 -> [128, nk*M] (K-tile blocks side by side)."""
    K, M = wt.shape
    nk = K // 128
    assert nk * 128 == K and M == block_cols
    return np.concatenate([wt[i * 128:(i + 1) * 128, :] for i in range(nk)], axis=1)


def _host_pack(points, emb_map, choose, cat_id, prior, params):
    """Build the 8 per-core input maps."""
    p = {k: [(np.asarray(w, np.float32), np.asarray(b, np.float32)) for w, b in v]
         for k, v in params.items()}

    def wT(key, i):
        return np.ascontiguousarray(p[key][i][0].T)

    wsm = np.zeros((128, 704), np.float32)
    wsm[0:3, 0:64] = wT("ig", 0)
    wsm[0:64, 64:128] = wT("ig", 1)
    wsm[0:64, 128:192] = wT("ig", 2)
    wsm[0:32, 192:256] = wT("ic", 0)
    wsm[0:3, 256:320] = wT("cl", 0)
    wsm[0:64, 320:384] = wT("cl", 1)
    wsm[0:64, 384:448] = wT("cl", 2)
    wsm[0:64, 448:576] = wT("cg", 0)
    wsm[0:128, 576:704] = wT("iglob", 0)

    wb = np.zeros((128, 40), np.float32)

    def put_bias(key, i, col, ncols):
        b = p[key][i][1]
        r = b.size // ncols
        wb[0:r, col:col + ncols] = b.reshape(ncols, r).T

    put_bias("ig", 0, 0, 1); put_bias("ig", 1, 1, 1); put_bias("ig", 2, 2, 1)
    put_bias("ic", 0, 3, 1)
    put_bias("cl", 0, 4, 1); put_bias("cl", 1, 5, 1); put_bias("cl", 2, 6, 1)
    put_bias("cg", 0, 7, 1); put_bias("iglob", 0, 8, 1)
    put_bias("t64", 0, 9, 4); put_bias("t64", 1, 13, 1)
    put_bias("t128", 0, 14, 4); put_bias("t128", 1, 18, 1)
    put_bias("assign", 0, 19, 4); put_bias("assign", 1, 23, 2)
    put_bias("deform", 0, 33, 4); put_bias("deform", 1, 37, 2)

    base = {
        "wsm": wsm,
        "wt64a": wT("t64", 0),
        "wt64b": _pack_blocks(wT("t64", 1), 64),
        "wt128a": _pack_blocks(wT("t128", 0), 512),
        "wt128b": _pack_blocks(wT("t128", 1), 128),
        "was0": wT("assign", 0),
        "was1": _pack_blocks(wT("assign", 1), 256),
        "wde0": wT("deform", 0),
        "wde1": _pack_blocks(wT("deform", 1), 256),
    }

    points = np.asarray(points, np.float32)
    prior = np.asarray(prior, np.float32)
    emb_map = np.asarray(emb_map, np.float32)
    choose = np.asarray(choose).astype(np.int64)
    cat_id = np.asarray(cat_id).astype(np.int64)

    was2_w = np.asarray(p["assign"][2][0], np.float32)   # (6144, 256)
    was2_b = np.asarray(p["assign"][2][1], np.float32)   # (6144,)
    wde2_w = np.asarray(p["deform"][2][0], np.float32)   # (18, 256)
    wde2_b = np.asarray(p["deform"][2][1], np.float32)   # (18,)

    in_maps = []
    for i in range(8):
        cat = int(cat_id[i])
        wbi = wb.copy()
        wbi[:, 25:33] = was2_b[cat * NV:(cat + 1) * NV].reshape(8, 128).T
        wbi[0:3, 39] = wde2_b[cat * 3:cat * 3 + 3]

        xin = np.empty((38, NPTS), np.float32)
        xin[0:3] = points[i].T
        xin[3:6] = prior[i].T
        xin[6:38] = emb_map[i][:, choose[i]]

        m = dict(base)
        m["wb"] = wbi
        m["xin"] = xin
        m["was2"] = _pack_blocks(
            np.ascontiguousarray(was2_w[cat * NV:(cat + 1) * NV, :].T), NV)
        m["wde2"] = _pack_blocks(
            np.ascontiguousarray(wde2_w[cat * 3:cat * 3 + 3, :].T), 3)
        in_maps.append(m)
    return in_maps


# Optional override used by test.py to run with NTFF profiling; the graded
# path never sets this.
RUNNER = None


def _assemble(results):
    assign = np.stack([results[i]["assign_T"].T for i in range(8)])
    deltas = np.stack([results[i]["deltas_T"].T for i in range(8)])
    return np.ascontiguousarray(assign), np.ascontiguousarray(deltas)


def kernel(points, emb_map, choose, cat_id, prior, params):
    nc = _get_program()
    in_maps = _host_pack(points, emb_map, choose, cat_id, prior, params)
    if RUNNER is not None:
        results = RUNNER(nc, in_maps)
    else:
        results = run_bass_kernel_spmd(nc, in_maps, list(range(8))).results
    return _assemble(results)


# revision 18
# speedup vs baseline: 1.1061x; 1.1061x over previous
"""Trainium2 Bass kernel for nn_DeformNet (dense per-point MLP network).

Strategy: pure data parallel over batch (bs=8 -> 1 batch item per NeuronCore).
All matmuls run in "channels on partitions" layout: activations are [C, n_pts]
tiles, weights are pre-transposed [Cin, Cout] (lhsT), so out = W @ x lands as
[Cout, n_pts] in PSUM. The per-sample category selection (cat_id) is applied
host-side by slicing the final assignment/deformation weight rows before
shipping them to the device (each core only computes the 1024 assignment
channels + 3 deformation channels its batch item actually needs). The big
[nv, n_pts] result is produced transposed on device and swapped on the host.

Matmuls use the float32r PE mode (TF32-like rounding, ~234 ns/matmul at
N=512 when warm). The instance-geometry and category-local 3-layer chains
are fused pairwise with block-diagonal weights so one matmul computes both.
PSUM drains are split between ScalarE and VectorE; a few fp32 dummy matmuls
at kernel start pre-warm the PE clock (HAM) before the real stream begins.
"""

import os
import sys

for _p in ("/opt/trn_rl_repo", "/root/.axon_site/_ro/trn_rl_repo"):
    if _p not in sys.path and os.path.isdir(_p):
        sys.path.append(_p)

import numpy as np

import concourse.bass as bass
import concourse.tile as tile
from concourse import bacc, mybir
from concourse.bass_utils import run_bass_kernel_spmd

F32 = mybir.dt.float32
F32R = mybir.dt.float32r
RELU = mybir.ActivationFunctionType.Relu
COPY = mybir.ActivationFunctionType.Identity
ADD = mybir.AluOpType.add
MAX = mybir.AluOpType.max

NPTS = 1024
NV = 1024
NCAT = 6
NHALF = 512  # fp32 moving-operand max per matmul

WSM_W = 744  # 704 weight cols + 37 bias cols (padded)
BIAS_BASE = 704

# bias column map (columns BIAS_BASE+c of the wsm tile)
BIAS_COLS = {
    "p1": 0, "p2": 1, "p3": 2,   # paired ig/cl biases: rows 0:64 ig, 64:128 cl
    "ic": 3, "cg": 4, "iglob": 5,
    "t64a": 6,    # 4 cols
    "t64b": 10,
    "t128a": 11,  # 4 cols
    "t128b": 15,
    "as0": 16,    # 4 cols
    "as1": 20,    # 2 cols
    "as2": 22,    # 8 cols
    "de0": 30,    # 4 cols
    "de1": 34,    # 2 cols
    "de2": 36,
}

_PROGRAM = None


def _build_program():
    nc = bacc.Bacc("TRN2", target_bir_lowering=False, debug=False, num_devices=8)

    dram_in = {}
    for name, shape in [
        ("xin", [38, NPTS]),
        ("wsm", [128, WSM_W]),
        ("wt64a", [128, 512]), ("wt64b", [128, 256]),
        ("wt128a", [128, 1024]), ("wt128b", [128, 512]),
        ("was0", [128, 512]), ("was1", [128, 1024]), ("was2", [128, 2048]),
        ("wde0", [128, 512]), ("wde1", [128, 1024]), ("wde2", [128, 6]),
    ]:
        dram_in[name] = nc.dram_tensor(name, shape, F32, kind="ExternalInput")

    out_a = nc.dram_tensor("assign_T", [NV, NPTS], F32, kind="ExternalOutput")
    out_d = nc.dram_tensor("deltas_T", [3, NPTS], F32, kind="ExternalOutput")

    with tile.TileContext(nc) as tc:
        with (
            tc.tile_pool(name="w", bufs=1) as wp,
            tc.tile_pool(name="a", bufs=1) as ap_,
            tc.tile_pool(name="hb", bufs=2) as hp,
            tc.tile_pool(name="ps", bufs=4, space="PSUM") as pp,
            tc.tile_pool(name="o", bufs=3) as op_,
        ):
            # ---- critical-path loads first on sync; big weights spread
            # across the other DMA-capable sequencers ----
            wsm_t = wp.tile([128, WSM_W], F32R, tag="wsm")
            nc.sync.dma_start(wsm_t[:], dram_in["wsm"][:].bitcast(F32R))
            xpp = wp.tile([6, NPTS], F32R, tag="xpp")
            xemb = wp.tile([32, NPTS], F32R, tag="xemb")
            nc.sync.dma_start(xpp[:], dram_in["xin"][0:6, :].bitcast(F32R))
            nc.sync.dma_start(xemb[:], dram_in["xin"][6:38, :].bitcast(F32R))

            # ---- PE warm-up: fp32 dummy matmuls (4 cyc/row -> dense PE
            # busy) prime the HAM activity window so the real matmul stream
            # runs at 2.4 GHz ----
            wz = wp.tile([128, NHALF], F32, tag="wz")
            nc.vector.memset(wz[:], 0.0)
            for i in range(5):
                psw = pp.tile([128, NHALF], F32, tag="ps")
                nc.tensor.matmul(psw[:], wz[:, 0:128], wz[:],
                                 start=True, stop=True)

            DMA_ENG = {
                "wt64a": nc.scalar, "wt64b": nc.scalar,
                "wt128a": nc.scalar, "wt128b": nc.scalar,
                "was0": nc.gpsimd, "wde0": nc.gpsimd,
                "was1": nc.gpsimd, "wde1": nc.gpsimd,
                "was2": nc.sync, "wde2": nc.sync,
            }
            W = {"wsm": wsm_t}
            for name in ("wt64a", "wt64b", "wt128a", "wt128b",
                         "was0", "wde0", "was1", "wde1", "was2", "wde2"):
                cols = dram_in[name].shape[1]
                t = wp.tile([128, cols], F32R, tag=name)
                DMA_ENG[name].dma_start(t[:], dram_in[name][:].bitcast(F32R))
                W[name] = t

            wsmb = wsm_t[:].bitcast(F32)

            def bias(key, rows, off=0, prow=0):
                c = BIAS_BASE + BIAS_COLS[key] + off
                return wsmb[prow:prow + rows, c:c + 1]

            def mm(ps, lhsT, rhs, start, stop):
                nc.tensor.matmul(ps, lhsT, rhs, start=start, stop=stop)

            def drain_act(ps, dst, b, relu=True):
                nc.scalar.activation(dst, ps, RELU if relu else COPY, bias=b)

            def drain_dve_relu(ps, dst, b):
                nc.vector.tensor_scalar(dst, ps, b, 0.0, ADD, MAX)

            def drain_dve_bias(ps, dst, b):
                nc.vector.tensor_scalar(dst, ps, b, None, ADD)

            def drain2(ps, dst, b, relu=True):
                # split the drain: ACT takes the first half, DVE the second,
                # so PSUM evacuation latency halves and both engines load
                h = NHALF
                drain_act(ps[:, 0:h], dst[:, 0:h], b, relu)
                if relu:
                    drain_dve_relu(ps[:, h:2 * h], dst[:, h:2 * h], b)
                else:
                    drain_dve_bias(ps[:, h:2 * h], dst[:, h:2 * h], b)

            # ---- fused instance-geometry (rows 0:64) + category-local
            # (rows 64:128) chains via block-diagonal weights ----
            ab1 = ap_.tile([128, NPTS], F32R, tag="ab1")
            ab2 = ap_.tile([128, NPTS], F32R, tag="ab2")
            bcl = ap_.tile([128, NPTS], F32R, tag="bcl")  # cl3 out in rows 64:128
            tpe = ap_.tile([128, NPTS], F32R, tag="tpe")
            ig = ap_.tile([128, NPTS], F32R, tag="ig")
            cg = ap_.tile([128, NPTS], F32R, tag="cg")

            def pair_layer(wcol, K, src, dst, bkey):
                ps = pp.tile([128, NPTS], F32, tag="ps")
                for n in range(2):
                    sl = slice(n * NHALF, (n + 1) * NHALF)
                    mm(ps[:, sl], wsm_t[0:K, wcol:wcol + 128], src[0:K, sl],
                       start=True, stop=True)
                return ps

            ps = pair_layer(0, 6, xpp[:], None, None)
            drain2(ps[:], ab1[:], bias("p1", 128))

            # ic is independent -> fills the PE while ab1 drains
            ps_ic = pp.tile([64, NPTS], F32, tag="ps")
            for n in range(2):
                sl = slice(n * NHALF, (n + 1) * NHALF)
                mm(ps_ic[:, sl], wsm_t[0:32, 384:448], xemb[:, sl],
                   start=True, stop=True)
            # partition-shifted write (psum rows 0:64 -> tpe rows 64:128):
            # keep it on ACT, which handles shifted outputs
            drain_act(ps_ic[:], tpe[64:128, :], bias("ic", 64))

            ps = pair_layer(128, 128, ab1[:], None, None)
            drain2(ps[:], ab2[:], bias("p2", 128))

            ps = pair_layer(256, 128, ab2[:], None, None)
            # rows 0:64 -> tpe (pts path), rows 64:128 -> bcl (cl3 out)
            drain_act(ps[0:64, :], tpe[0:64, :], bias("p3", 64))
            drain_dve_relu(ps[64:128, :], bcl[64:128, :], bias("p3", 64, prow=64))

            # cg: weights placed at wsm rows 64:128 so lhsT/rhs base match
            ps = pp.tile([128, NPTS], F32, tag="ps")
            for n in range(2):
                sl = slice(n * NHALF, (n + 1) * NHALF)
                mm(ps[:, sl], wsm_t[64:128, 448:576], bcl[64:128, sl],
                   start=True, stop=True)
            drain2(ps[:], cg[:], bias("cg", 128))

            # ---- t64 relation: (128 -> 512 relu -> 64) on tpe ----
            h64 = hp.tile([128, 4, NPTS], F32R, tag="hbig")
            for m in range(4):
                ps = pp.tile([128, NPTS], F32, tag="ps")
                for n in range(2):
                    sl = slice(n * NHALF, (n + 1) * NHALF)
                    mm(ps[:, sl], W["wt64a"][:, m * 128:(m + 1) * 128],
                       tpe[:, sl], start=True, stop=True)
                drain2(ps[:], h64[:, m, :], bias("t64a", 128, m))

            # p duplicated into both halves of a [128, n] tile so the
            # residual add is one full-width tensor_tensor
            psb = ap_.tile([128, NPTS], F32, tag="psb")
            ps = pp.tile([64, NPTS], F32, tag="ps")
            for n in range(2):
                sl = slice(n * NHALF, (n + 1) * NHALF)
                for k in range(4):
                    mm(ps[:, sl], W["wt64b"][:, k * 64:(k + 1) * 64],
                       h64[:, k, sl], start=(k == 0), stop=(k == 3))
            drain_dve_bias(ps[:], psb[0:64, :], bias("t64b", 64))
            drain_act(ps[:], psb[64:128, :], bias("t64b", 64), relu=False)
            nc.vector.tensor_add(tpe[:].bitcast(F32R),
                                 tpe[:].bitcast(F32), psb[:])

            # ---- inst_global ----
            ps = pp.tile([128, NPTS], F32, tag="ps")
            for n in range(2):
                sl = slice(n * NHALF, (n + 1) * NHALF)
                mm(ps[:, sl], wsm_t[:, 576:704], tpe[:, sl], start=True, stop=True)
            drain2(ps[:], ig[:], bias("iglob", 128))

            # ---- t128 relation: (256 -> 512 relu -> 128) on [ig; cg] ----
            h128 = hp.tile([128, 4, NPTS], F32R, tag="hbig")
            for m in range(4):
                ps = pp.tile([128, NPTS], F32, tag="ps")
                for n in range(2):
                    sl = slice(n * NHALF, (n + 1) * NHALF)
                    for k, src in ((0, ig), (1, cg)):
                        mm(ps[:, sl],
                           W["wt128a"][:, k * 512 + m * 128:k * 512 + (m + 1) * 128],
                           src[:, sl], start=(k == 0), stop=(k == 1))
                drain2(ps[:], h128[:, m, :], bias("t128a", 128, m))

            qsb = ap_.tile([128, NPTS], F32, tag="qsb")
            ps = pp.tile([128, NPTS], F32, tag="ps")
            for n in range(2):
                sl = slice(n * NHALF, (n + 1) * NHALF)
                for k in range(4):
                    mm(ps[:, sl], W["wt128b"][:, k * 128:(k + 1) * 128],
                       h128[:, k, sl], start=(k == 0), stop=(k == 3))
            drain2(ps[:], qsb[:], bias("t128b", 128), relu=False)
            # ig add on DVE (gates as0, the critical path); cg add runs in
            # parallel on GpSimd (only gates de0)
            nc.vector.tensor_add(ig[:].bitcast(F32R), ig[:].bitcast(F32), qsb[:])
            nc.gpsimd.tensor_add(cg[:].bitcast(F32R), cg[:].bitcast(F32), qsb[:])

            # ---- assign & deform heads ----
            ah1 = hp.tile([128, 4, NPTS], F32R, tag="hbig")
            dh1 = hp.tile([128, 4, NPTS], F32R, tag="hbig")
            ah2 = hp.tile([128, 2, NPTS], F32R, tag="h2")
            dh2 = hp.tile([128, 2, NPTS], F32R, tag="h2")

            def head_l1_tile(wkey, src, dst, bkey, m):
                ps = pp.tile([128, NPTS], F32, tag="ps")
                for n in range(2):
                    sl = slice(n * NHALF, (n + 1) * NHALF)
                    mm(ps[:, sl], W[wkey][:, m * 128:(m + 1) * 128],
                       src[:, sl], start=True, stop=True)
                drain2(ps[:], dst[:, m, :], bias(bkey, 128, m))

            def head_l2_tile(wkey, src, dst, bkey, m):
                ps = pp.tile([128, NPTS], F32, tag="ps")
                for n in range(2):
                    sl = slice(n * NHALF, (n + 1) * NHALF)
                    for k in range(4):
                        mm(ps[:, sl],
                           W[wkey][:, k * 256 + m * 128:k * 256 + (m + 1) * 128],
                           src[:, k, sl], start=(k == 0), stop=(k == 3))
                drain2(ps[:], dst[:, m, :], bias(bkey, 128, m))

            # assign head first so its big output DMAs start as early as
            # possible; de0/de1 fill the PE while as0/as1 results drain
            for m in range(4):
                head_l1_tile("was0", ig, ah1, "as0", m)
            for m in range(4):
                head_l1_tile("wde0", cg, dh1, "de0", m)
            for m in range(2):
                head_l2_tile("was1", ah1, ah2, "as1", m)
            for m in range(2):
                head_l2_tile("wde1", dh1, dh2, "de1", m)

            def as2_tile(m):
                ps = pp.tile([128, NPTS], F32, tag="ps")
                for n in range(2):
                    sl = slice(n * NHALF, (n + 1) * NHALF)
                    for k in range(2):
                        mm(ps[:, sl],
                           W["was2"][:, k * 1024 + m * 128:k * 1024 + (m + 1) * 128],
                           ah2[:, k, sl], start=(k == 0), stop=(k == 1))
                ot = op_.tile([128, NPTS], F32, tag="oa")
                # full-width drain, alternating engines (throughput over
                # latency here)
                if m % 2 == 0:
                    drain_act(ps[:], ot[:], bias("as2", 128, m), relu=False)
                else:
                    drain_dve_bias(ps[:], ot[:], bias("as2", 128, m))
                nc.sync.dma_start(out_a[m * 128:(m + 1) * 128, :], ot[:])

            as2_tile(0)
            as2_tile(1)

            # tiny deform head slots in behind the first output tiles
            ps = pp.tile([3, NPTS], F32, tag="ps")
            for n in range(2):
                sl = slice(n * NHALF, (n + 1) * NHALF)
                for k in range(2):
                    mm(ps[:, sl], W["wde2"][:, k * 3:(k + 1) * 3],
                       dh2[:, k, sl], start=(k == 0), stop=(k == 1))
            od = op_.tile([3, NPTS], F32, tag="od")
            drain_dve_bias(ps[:], od[:], bias("de2", 3))
            nc.sync.dma_start(out_d[:], od[:])

            for m in range(2, 8):
                as2_tile(m)

    nc.compile()
    return nc


def _get_program():
    global _PROGRAM
    if _PROGRAM is None:
        _PROGRAM = _build_program()
    return _PROGRAM


def _pack_blocks(wt, block_cols):
    """[K, M] with K = nk*128 -> [128, nk*M] (K-tile blocks side by side)."""
    K, M = wt.shape
    nk = K // 128
    assert nk * 128 == K and M == block_cols
    return np.concatenate([wt[i * 128:(i + 1) * 128, :] for i in range(nk)], axis=1)


def _host_pack(points, emb_map, choose, cat_id, prior, params):
    """Build the 8 per-core input maps."""
    p = {k: [(np.asarray(w, np.float32), np.asarray(b, np.float32)) for w, b in v]
         for k, v in params.items()}

    def wT(key, i):
        return np.ascontiguousarray(p[key][i][0].T)

    wsm = np.zeros((128, WSM_W), np.float32)
    # block-diagonal pairs: rows 0:K_ig cols 0:64 = ig_i, rows K.. cols 64:128 = cl_i
    wsm[0:3, 0:64] = wT("ig", 0)
    wsm[3:6, 64:128] = wT("cl", 0)
    wsm[0:64, 128:192] = wT("ig", 1)
    wsm[64:128, 192:256] = wT("cl", 1)
    wsm[0:64, 256:320] = wT("ig", 2)
    wsm[64:128, 320:384] = wT("cl", 2)
    wsm[0:32, 384:448] = wT("ic", 0)
    wsm[64:128, 448:576] = wT("cg", 0)   # rows 64:128: rhs lives there too
    wsm[0:128, 576:704] = wT("iglob", 0)

    def put_bias(vec, col, prow=0):
        vec = np.asarray(vec, np.float32).reshape(-1)
        wsm[prow:prow + vec.size, BIAS_BASE + col] = vec

    put_bias(np.concatenate([p["ig"][0][1], p["cl"][0][1]]), BIAS_COLS["p1"])
    put_bias(np.concatenate([p["ig"][1][1], p["cl"][1][1]]), BIAS_COLS["p2"])
    put_bias(np.concatenate([p["ig"][2][1], p["cl"][2][1]]), BIAS_COLS["p3"])
    put_bias(p["ic"][0][1], BIAS_COLS["ic"])
    put_bias(p["cg"][0][1], BIAS_COLS["cg"])
    put_bias(p["iglob"][0][1], BIAS_COLS["iglob"])

    def put_bias_tiles(key, i, col):
        b = p[key][i][1]
        n = b.size // 128
        for j in range(n):
            put_bias(b[j * 128:(j + 1) * 128], col + j)

    put_bias_tiles("t64", 0, BIAS_COLS["t64a"])
    put_bias(p["t64"][1][1], BIAS_COLS["t64b"])
    put_bias_tiles("t128", 0, BIAS_COLS["t128a"])
    put_bias(p["t128"][1][1], BIAS_COLS["t128b"])
    put_bias_tiles("assign", 0, BIAS_COLS["as0"])
    put_bias_tiles("assign", 1, BIAS_COLS["as1"])
    put_bias_tiles("deform", 0, BIAS_COLS["de0"])
    put_bias_tiles("deform", 1, BIAS_COLS["de1"])

    base = {
        "wt64a": wT("t64", 0),
        "wt64b": _pack_blocks(wT("t64", 1), 64),
        "wt128a": _pack_blocks(wT("t128", 0), 512),
        "wt128b": _pack_blocks(wT("t128", 1), 128),
        "was0": wT("assign", 0),
        "was1": _pack_blocks(wT("assign", 1), 256),
        "wde0": wT("deform", 0),
        "wde1": _pack_blocks(wT("deform", 1), 256),
    }

    points = np.asarray(points, np.float32)
    prior = np.asarray(prior, np.float32)
    emb_map = np.asarray(emb_map, np.float32)
    choose = np.asarray(choose).astype(np.int64)
    cat_id = np.asarray(cat_id).astype(np.int64)

    was2_w = np.asarray(p["assign"][2][0], np.float32)   # (6144, 256)
    was2_b = np.asarray(p["assign"][2][1], np.float32)   # (6144,)
    wde2_w = np.asarray(p["deform"][2][0], np.float32)   # (18, 256)
    wde2_b = np.asarray(p["deform"][2][1], np.float32)   # (18,)

    in_maps = []
    for i in range(8):
        cat = int(cat_id[i])
        wsmi = wsm.copy()
        b8 = was2_b[cat * NV:(cat + 1) * NV].reshape(8, 128)
        for j in range(8):
            wsmi[0:128, BIAS_BASE + BIAS_COLS["as2"] + j] = b8[j]
        wsmi[0:3, BIAS_BASE + BIAS_COLS["de2"]] = wde2_b[cat * 3:cat * 3 + 3]

        xin = np.empty((38, NPTS), np.float32)
        xin[0:3] = points[i].T
        xin[3:6] = prior[i].T
        xin[6:38] = emb_map[i][:, choose[i]]

        m = dict(base)
        m["wsm"] = wsmi
        m["xin"] = xin
        m["was2"] = _pack_blocks(
            np.ascontiguousarray(was2_w[cat * NV:(cat + 1) * NV, :].T), NV)
        m["wde2"] = _pack_blocks(
            np.ascontiguousarray(wde2_w[cat * 3:cat * 3 + 3, :].T), 3)
        in_maps.append(m)
    return in_maps


# Optional override used by test.py to run with NTFF profiling; the graded
# path never sets this.
RUNNER = None


def _assemble(results):
    assign = np.stack([results[i]["assign_T"].T for i in range(8)])
    deltas = np.stack([results[i]["deltas_T"].T for i in range(8)])
    return np.ascontiguousarray(assign), np.ascontiguousarray(deltas)


def kernel(points, emb_map, choose, cat_id, prior, params):
    nc = _get_program()
    in_maps = _host_pack(points, emb_map, choose, cat_id, prior, params)
    if RUNNER is not None:
        results = RUNNER(nc, in_maps)
    else:
        results = run_bass_kernel_spmd(nc, in_maps, list(range(8))).results
    return _assemble(results)
